# revision 5
# baseline (speedup 1.0000x reference)
"""Fused multi-head attention block (B=2, N=4096, C=768, H=12, D=64) for 8
Trainium2 NeuronCores.

Sharding: core c -> (batch b = c // 4, head-group g = c % 4, heads
[3g, 3g+1, 3g+2]).  Megatron-style: qkv weights column-split per head
group, proj weights row-split; each core emits a partial [N, C] output
and the host sums the 4 partials per batch and adds proj_b.

Per-core kernel v3 (flat-pipeline design):
  - x^T and all weights SBUF-resident in bf16.
  - One global software pipeline over all (chunk, head) attention
    streams: per 512-query x 384-key tile, PE emits S(g) then AV(g-2),
    with the AV drain of stream k interleaved with the first S tiles of
    stream k+1 (oacc ring of 2 PSUM banks) -- no PE bubble at head
    boundaries.
  - exp(S) split across THREE engines: ACT (exact exp instruction) for
    most tiles, DVE + GPSIMD via one-instruction Schraudolph
    (int16 <- A*S + B; the int16 IS the bf16 bit pattern of ~exp(S),
    max rel err ~3% sawtooth which largely cancels in the softmax
    ratio; end-to-end rel err ~1.25e-2, gate 2e-2).  ACT alone was the
    456us critical path in v2.
  - V^T never materialized: V is projected directly in [keys, dims]
    layout (lhsT = x^T block, rhs = W_v columns) into vaug, killing the
    PE transpose passes and their DVE copybacks.
  - Softmax denominators: ones-column 64 of vaug -> DVE reciprocal ->
    PE broadcast (f32r outer product) -> DVE multiply; the broadcast
    PSUM tile shares the oacc ring.
  - qproj / yproj / normalize / phase-1 slices are woven one-per-tile
    into the pipeline as 'extras' to fill the exp-latency gap on PE.
"""

import sys

sys.path.insert(0, "/opt/trn_rl_repo")

from collections import deque
from contextlib import ExitStack

import numpy as np
import ml_dtypes

import concourse.bacc as bacc
import concourse.bass as bass
import concourse.mybir as mybir
import concourse.tile as tile

B, N, C, H, D = 2, 4096, 768, 12, 64
SCALE = D ** -0.5
F32 = mybir.dt.float32
F32R = mybir.dt.float32r
BF16 = mybir.dt.bfloat16
I16 = mybir.dt.int16
BF16NP = ml_dtypes.bfloat16

EXPB = -2.0  # exp(S + EXPB): softmax-invariant shift
LOG2E = 1.4426950408889634
C_ADJ = 0.04305  # Schraudolph minimax centering (HW f32->i16 rounds)
A16 = (1 << 23) * LOG2E / 65536.0
B16 = (127.0 - C_ADJ) * 128.0 + A16 * EXPB

# column layout of wqkv (output dims of the projection):
# m0 q01 (q_ha|q_hb) 0:128 | m1 k01 128:256 | m2 [q_hc|q_hc] 256:384
# m3 k2 384:448 | m4 v01 448:576 | m5 v2 576:640
MOFF = [0, 128, 256, 384, 448, 576]
MW = [128, 128, 128, 64, 128, 64]

ADD = mybir.AluOpType.add
MULT = mybir.AluOpType.mult


def build_nc(seq=N, n_dve=3, n_pool=0):
    # n_pool must stay 0: GPSIMD cannot read PSUM (BIR verifier).
    NS = seq // 512   # 512-wide query chunks
    NB = seq // 128   # 128-wide key blocks
    TILES = []
    b = 0
    while b < NB:
        n = min(3, NB - b)
        if NB - b == 4:
            n = 2  # avoid a trailing 1-block tile
        TILES.append(list(range(b, b + n)))
        b += n
    NT = len(TILES)

    # per-stream exp engine assignment: spread DVE/Pool tiles over t
    n_off = min(n_dve + n_pool, NT)
    off_ts = [int((i + 0.5) * NT / n_off) for i in range(n_off)] if n_off else []
    dve_set = set(off_ts[:n_dve])
    pool_set = set(off_ts[n_dve:])

    nc = bacc.Bacc("TRN2", target_bir_lowering=False, debug=False, num_devices=8)
    xt = nc.dram_tensor("xt", [768, seq], BF16, kind="ExternalInput").ap()
    wqkv = nc.dram_tensor("wqkv", [768, 640], BF16, kind="ExternalInput").ap()
    wb = nc.dram_tensor("wb", [128, 6], F32, kind="ExternalInput").ap()
    pwt = nc.dram_tensor("pwt", [384, 768], BF16, kind="ExternalInput").ap()
    vbias = nc.dram_tensor("vbias", [128, 192], F32, kind="ExternalInput").ap()
    y = nc.dram_tensor("y", [seq, 768], F32, kind="ExternalOutput").ap()

    with tile.TileContext(nc) as tc, ExitStack() as ctx:
        const = ctx.enter_context(tc.tile_pool(name="const", bufs=1))
        big = ctx.enter_context(tc.tile_pool(name="big", bufs=1))
        pt_pool = ctx.enter_context(tc.tile_pool(name="ptp", bufs=3))
        osb_pool = ctx.enter_context(tc.tile_pool(name="osb", bufs=3))
        rr_pool = ctx.enter_context(tc.tile_pool(name="rrp", bufs=2))
        ysb_pool = ctx.enter_context(tc.tile_pool(name="ysb", bufs=2))
        stp = ctx.enter_context(tc.tile_pool(name="stp", bufs=2, space="PSUM"))
        oax = ctx.enter_context(tc.tile_pool(name="oax", bufs=2, space="PSUM"))

        # ---- weights / constants (issued on sync queue) ----
        w_sb = []
        for cch in range(6):
            t = const.tile([128, 640], BF16, tag=f"w{cch}", name=f"w{cch}")
            nc.sync.dma_start(t[:], wqkv[cch * 128:(cch + 1) * 128, :])
            w_sb.append(t)
        wb_sb = const.tile([128, 6], F32, tag="wb")
        nc.sync.dma_start(wb_sb[:], wb[:])
        vb_sb = const.tile([128, 192], F32, tag="vb")
        nc.sync.dma_start(vb_sb[:], vbias[:])

        # x^T resident: xst[cch][s] = [128, 512] bf16, s-major (issued on
        # gpsimd queue: cheap issue, keeps SP free)
        xst = [[None] * NS for _ in range(6)]
        for s in range(NS):
            for cch in range(6):
                t = const.tile([128, 512], BF16, tag=f"x{cch}_{s}", name="xs")
                nc.gpsimd.dma_start(
                    t[:], xt[cch * 128:(cch + 1) * 128, s * 512:(s + 1) * 512])
                xst[cch][s] = t
            if s == 1:
                pw_sb = [const.tile([128, 768], BF16, tag=f"pw{h}", name=f"pwt{h}")
                         for h in range(3)]
                for h in range(3):
                    nc.sync.dma_start(pw_sb[h][:], pwt[h * 128:(h + 1) * 128, :])

        ones_sb = const.tile([128, 64], F32R, tag="ones")
        nc.vector.memset(ones_sb[:].bitcast(F32), 1.0)
        expb_sb = const.tile([128, 1], F32, tag="expb")
        nc.vector.memset(expb_sb[:], EXPB)

        # ---- persistent tensors ----
        q01 = big.tile([128, seq], BF16, tag="q01")
        q2 = big.tile([128, seq], BF16, tag="q2")
        ka = big.tile([128, seq], BF16, tag="ka")
        kb = big.tile([128, seq], BF16, tag="kb")
        kc = big.tile([128, seq], BF16, tag="kc")
        nc.gpsimd.memset(ka[64:128, :].bitcast(mybir.dt.uint16), 0)
        nc.gpsimd.memset(kb[0:64, :].bitcast(mybir.dt.uint16), 0)
        nc.gpsimd.memset(kc[64:128, :].bitcast(mybir.dt.uint16), 0)
        # vaug[:, h, blk, 0:64] = V block; col 64 = 1.0 (denominator row)
        vaug = big.tile([128, 3, NB, 80], BF16, tag="va", name="va")
        nc.gpsimd.memset(vaug[:], 1.0)
        otp = [big.tile([128, 512], BF16, tag=f"otp{h}", name=f"otp{h}")
               for h in range(3)]
        for h in range(3):
            nc.gpsimd.memset(otp[h][:].bitcast(mybir.dt.uint16), 0)

        heads = [(ka, q01), (kb, q01), (kc, q2)]
        streams = [(Q, h) for Q in range(NS) for h in range(3)]
        NSTR = len(streams)

        def copyback(dst, srcap, bias_ap):
            nc.vector.tensor_scalar_add(dst, srcap, bias_ap)

        # ---------- emission helpers ----------

        def emit_qproj_slice(Qn, mi):
            """mi 0 -> q01, 1 -> q2 (m-group 0 / 2)."""
            m = (0, 2)[mi]
            ps = stp.tile([128, 512], F32, tag="stp", name="psq")
            for cch in range(6):
                nc.tensor.matmul(
                    ps[:],
                    lhsT=w_sb[cch][:, MOFF[m]:MOFF[m] + 128],
                    rhs=xst[cch][Qn][:],
                    start=(cch == 0),
                    stop=(cch == 5),
                )
            qs = slice(Qn * 512, (Qn + 1) * 512)
            dst = (q01, q2)[mi]
            copyback(dst[:, qs], ps[:], wb_sb[:, (0, 2)[mi]:(0, 2)[mi] + 1])

        def emit_k01(s):
            ss = slice(s * 512, (s + 1) * 512)
            ps = stp.tile([128, 512], F32, tag="stp", name="psk")
            for cch in range(6):
                nc.tensor.matmul(
                    ps[:],
                    lhsT=w_sb[cch][:, MOFF[1]:MOFF[1] + 128],
                    rhs=xst[cch][s][:],
                    start=(cch == 0),
                    stop=(cch == 5),
                )
            copyback(ka[0:64, ss], ps[0:64, :], wb_sb[0:64, 1:2])
            copyback(kb[64:128, ss], ps[64:128, :], wb_sb[64:128, 1:2])

        def emit_k2(s):
            ss = slice(s * 512, (s + 1) * 512)
            ps = stp.tile([128, 512], F32, tag="stp", name="psk2")
            for cch in range(6):
                nc.tensor.matmul(
                    ps[0:64, :],
                    lhsT=w_sb[cch][:, MOFF[3]:MOFF[3] + 64],
                    rhs=xst[cch][s][:],
                    start=(cch == 0),
                    stop=(cch == 5),
                )
            copyback(kc[0:64, ss], ps[0:64, :], wb_sb[0:64, 3:4])

        def emit_vdir(s, j):
            """V projected directly as [128 keys, 192 dims(3 heads)]."""
            ps = stp.tile([128, 192], F32, tag="stp", name="psv")
            for cch in range(6):
                nc.tensor.matmul(
                    ps[:],
                    lhsT=xst[cch][s][:, j * 128:(j + 1) * 128],
                    rhs=w_sb[cch][:, 448:640],
                    start=(cch == 0),
                    stop=(cch == 5),
                )
            blk = 4 * s + j
            # free orders line up: ps/vb cols = [va|vb|vc], out = h-major
            nc.vector.tensor_tensor(
                vaug[:, :, blk:blk + 1, 0:64], ps[:], vb_sb[:], ADD)

        def emit_exp(ps, pt, nl, eng):
            if eng == "act":
                nc.scalar.activation(
                    pt[:, 0:nl, :], ps[:, 0:nl, :],
                    mybir.ActivationFunctionType.Exp, bias=expb_sb[:],
                )
            else:
                e = nc.vector if eng == "dve" else nc.gpsimd
                e.tensor_scalar(
                    pt[:, 0:nl, :].bitcast(I16), ps[:, 0:nl, :],
                    float(A16), float(B16), MULT, ADD,
                )

        def emit_av(oacc, h, pt, blocks, start, stop):
            n = len(blocks)
            for j, blk in enumerate(blocks):
                nc.tensor.matmul(
                    oacc[:],
                    lhsT=vaug[:, h:h + 1, blk:blk + 1, 0:65],
                    rhs=pt[:, j:j + 1, :],
                    start=(start and j == 0),
                    stop=(stop and j == n - 1),
                )

        def emit_yproj(Q, nt):
            ps = stp.tile([128, 2, 512], F32, tag="stp", name="psy")
            for sl, co, cw in ((0, 0, 512), (1, 512, 256)):
                for h in range(3):
                    nc.tensor.matmul(
                        ps[:, sl:sl + 1, 0:cw],
                        lhsT=otp[h][:, nt * 128:(nt + 1) * 128],
                        rhs=pw_sb[h][:, co:co + cw],
                        start=(h == 0),
                        stop=(h == 2),
                    )
            ysb = ysb_pool.tile([128, 768], F32, tag="ysb", name="ysb")
            nc.vector.tensor_copy(ysb[:, 0:512], ps[:, 0:1, :])
            nc.vector.tensor_copy(ysb[:, 512:768], ps[:, 1:2, 0:256])
            r0 = Q * 512 + nt * 128
            nc.sync.dma_start(y[r0:r0 + 128, :], ysb[:])

        def emit_norm(h, osb, rrr):
            psb = oax.tile([64, 512], F32, tag="o", name="psb")
            nc.tensor.matmul(
                psb[0:64, :],
                lhsT=ones_sb[64:65, 0:64],
                rhs=rrr[64:65, :],
                start=True,
                stop=True,
            )
            nc.vector.tensor_mul(otp[h][0:64, :], osb[0:64, :], psb[0:64, :])

        # ---------- flat pipeline driver ----------

        extras = deque()   # (kind, Q, closure)
        pend = deque()     # (si, t, pt)
        oaccs = {}

        def pop_extra():
            if extras:
                extras.popleft()[2]()

        def run_qproj_now(Q):
            """Force any still-queued qproj(Q) extras before stream (Q, 0)."""
            keep = deque()
            while extras:
                e = extras.popleft()
                if e[0] == "qproj" and e[1] == Q:
                    e[2]()
                else:
                    keep.append(e)
            extras.extend(keep)

        def emit_stream_tile(si, t):
            Q, h = streams[si]
            kt, qt = heads[h]
            blocks = TILES[t]
            nl = len(blocks)
            qs = slice(Q * 512, (Q + 1) * 512)
            ps = stp.tile([128, nl, 512], F32, tag="stp", name="ps")
            for j, blk in enumerate(blocks):
                nc.tensor.matmul(
                    ps[:, j:j + 1, :],
                    lhsT=kt[:, blk * 128:(blk + 1) * 128],
                    rhs=qt[:, qs],
                    start=True,
                    stop=True,
                )
            pt = pt_pool.tile([128, 3, 512], BF16, tag="pt", name="pt")
            if si == 0:
                eng = "act"  # phase-1: DVE/Pool busy with copybacks
            else:
                eng = "dve" if t in dve_set else ("pool" if t in pool_set else "act")
            emit_exp(ps, pt, nl, eng)
            pend.append((si, t, pt))

        def retire_av():
            si, t, pt = pend.popleft()
            Q, h = streams[si]
            if t == 0:
                oaccs[si] = oax.tile([65, 512], F32, tag="o", name="oacc")
            emit_av(oaccs[si], h, pt, TILES[t], start=(t == 0), stop=(t == NT - 1))
            if t == NT - 1:
                end_stream(si)

        def end_stream(si):
            Q, h = streams[si]
            oacc = oaccs.pop(si)
            osb = osb_pool.tile([65, 512], F32, tag="osb", name="osb")
            nc.vector.tensor_copy(osb[:], oacc[:])
            rr = rr_pool.tile([65, 512], F32, tag="rr", name="rr")
            nc.vector.reciprocal(rr[64:65, :], osb[64:65, :])
            rrr = rr_pool.tile([65, 512], F32R, tag="rrr", name="rrr")
            nc.vector.tensor_copy(rrr[64:65, :], rr[64:65, :])
            extras.append(("norm", Q, lambda h=h, o=osb, r=rrr: emit_norm(h, o, r)))
            if h == 2:
                for nt in range(4):
                    extras.append(
                        ("yproj", Q, lambda Q=Q, nt=nt: emit_yproj(Q, nt)))
            if h == 1 and Q + 1 < NS:
                for mi in range(2):
                    extras.append(
                        ("qproj", Q + 1,
                         lambda Q1=Q + 1, mi=mi: emit_qproj_slice(Q1, mi)))

        # phase 0: q projection for chunk 0
        emit_qproj_slice(0, 0)
        emit_qproj_slice(0, 1)

        # phase 1: k/v projection for all chunks, interleaved with stream 0
        t_next = 0

        def pull_stream0(s_done):
            nonlocal t_next
            while t_next < NT and TILES[t_next][-1] < 4 * s_done:
                emit_stream_tile(0, t_next)
                t_next += 1
                if len(pend) > 2:
                    retire_av()

        for s in range(NS):
            emit_k01(s)
            pull_stream0(s)      # keys of chunk s-1 usable
            emit_k2(s)
            for j in range(4):
                emit_vdir(s, j)
                pull_stream0(s)
        pull_stream0(NS)         # all keys ready: rest of stream 0

        # steady state: remaining streams
        for si in range(1, NSTR):
            Q, h = streams[si]
            if h == 0:
                run_qproj_now(Q)
            for t in range(NT):
                emit_stream_tile(si, t)
                if len(pend) > 2:
                    retire_av()
                if t >= 1:
                    pop_extra()

        # tail: drain pipeline and remaining extras
        while pend:
            retire_av()
        while extras:
            pop_extra()

    nc.compile()
    return nc


def host_prep(x, qkv_w, qkv_b, proj_w, seq=N):
    """Build the 8 per-core input maps."""
    f = np.float32
    x = np.asarray(x, f)
    qkv_w = np.asarray(qkv_w, f)
    qkv_b = np.asarray(qkv_b, f)
    proj_w = np.asarray(proj_w, f)

    xts = [np.ascontiguousarray(x[b].T).astype(BF16NP) for b in range(B)]

    in_maps = []
    for core in range(8):
        b, g = core // 4, core % 4
        ha, hb_, hc = 3 * g, 3 * g + 1, 3 * g + 2

        def Wrow(base, h):
            return qkv_w[base + h * 64: base + (h + 1) * 64, :]  # [64, 768]

        def brow(base, h):
            return qkv_b[base + h * 64: base + (h + 1) * 64]

        cols = np.concatenate(
            [
                Wrow(0, ha).T * SCALE, Wrow(0, hb_).T * SCALE,   # q01
                Wrow(C, ha).T, Wrow(C, hb_).T,                   # k01 -> ka/kb
                Wrow(0, hc).T * SCALE, Wrow(0, hc).T * SCALE,    # q2 duplicated
                Wrow(C, hc).T,                                   # k2
                Wrow(2 * C, ha).T, Wrow(2 * C, hb_).T,           # v01
                Wrow(2 * C, hc).T,                               # v2
            ],
            axis=1,
        )  # [768, 640]
        bias = np.concatenate(
            [
                brow(0, ha) * SCALE, brow(0, hb_) * SCALE,
                brow(C, ha), brow(C, hb_),
                brow(0, hc) * SCALE, brow(0, hc) * SCALE,
                brow(C, hc),
                brow(2 * C, ha), brow(2 * C, hb_), brow(2 * C, hc),
            ]
        )  # [640]
        wbm = np.zeros((128, 6), f)
        for m in range(6):
            wbm[0:MW[m], m] = bias[MOFF[m]:MOFF[m] + MW[m]]
        vbm = np.zeros((128, 192), f)
        vbm[:, :] = bias[448:640][None, :]
        pwt = np.zeros((384, 768), f)
        for i, h in enumerate((ha, hb_, hc)):
            pwt[i * 128:i * 128 + 64, :] = proj_w.T[h * 64:(h + 1) * 64, :]

        in_maps.append(
            {
                "xt": xts[b][:, :seq],
                "wqkv": np.ascontiguousarray(cols).astype(BF16NP),
                "wb": wbm,
                "vbias": vbm,
                "pwt": pwt.astype(BF16NP),
            }
        )
    return in_maps


_nc_cache = {}


def _get_nc(seq=N):
    key = (seq,)
    if key not in _nc_cache:
        _nc_cache[key] = build_nc(seq)
    return _nc_cache[key]


def kernel(x, qkv_w, qkv_b, proj_w, proj_b, _trace=False):
    from concourse.bass_utils import run_bass_kernel_spmd

    nc = _get_nc()
    in_maps = host_prep(x, qkv_w, qkv_b, proj_w)
    res = run_bass_kernel_spmd(nc, in_maps, list(range(8)), trace=_trace)
    proj_b = np.asarray(proj_b, np.float32)
    out = np.zeros((B, N, C), np.float32)
    for b in range(B):
        acc = np.zeros((N, C), np.float32)
        for g in range(4):
            acc += res.results[b * 4 + g]["y"]
        out[b] = acc + proj_b[None, :]
    if _trace:
        return out, res
    return out


# revision 11
# speedup vs baseline: 1.0611x; 1.0611x over previous
"""Fused multi-head attention block (B=2, N=4096, C=768, H=12, D=64) for 8
Trainium2 NeuronCores.

Sharding: core c -> (batch b = c // 4, head-group g = c % 4, heads
[3g, 3g+1, 3g+2]).  Megatron-style: qkv weights column-split per head
group, proj weights row-split; each core emits a partial [N, C] output
and the host sums the 4 partials per batch and adds proj_b.

Per-core kernel v3 (flat-pipeline design):
  - x^T and all weights SBUF-resident in bf16.
  - One global software pipeline over all (chunk, head) attention
    streams: per 512-query x 384-key tile, PE emits S(g) then AV(g-2),
    with the AV drain of stream k interleaved with the first S tiles of
    stream k+1 (oacc ring of 2 PSUM banks) -- no PE bubble at head
    boundaries.
  - exp(S) split across THREE engines: ACT (exact exp instruction) for
    most tiles, DVE + GPSIMD via one-instruction Schraudolph
    (int16 <- A*S + B; the int16 IS the bf16 bit pattern of ~exp(S),
    max rel err ~3% sawtooth which largely cancels in the softmax
    ratio; end-to-end rel err ~1.25e-2, gate 2e-2).  ACT alone was the
    456us critical path in v2.
  - V^T never materialized: V is projected directly in [keys, dims]
    layout (lhsT = x^T block, rhs = W_v columns) into vaug, killing the
    PE transpose passes and their DVE copybacks.
  - Softmax denominators: ones-column 64 of vaug -> DVE reciprocal ->
    PE broadcast (f32r outer product) -> DVE multiply; the broadcast
    PSUM tile shares the oacc ring.
  - qproj / yproj / normalize / phase-1 slices are woven one-per-tile
    into the pipeline as 'extras' to fill the exp-latency gap on PE.
"""

import sys

sys.path.insert(0, "/opt/trn_rl_repo")

from collections import deque
from contextlib import ExitStack

import numpy as np
import ml_dtypes

import concourse.bacc as bacc
import concourse.bass as bass
import concourse.mybir as mybir
import concourse.tile as tile

B, N, C, H, D = 2, 4096, 768, 12, 64
SCALE = D ** -0.5
F32 = mybir.dt.float32
F32R = mybir.dt.float32r
BF16 = mybir.dt.bfloat16
I16 = mybir.dt.int16
BF16NP = ml_dtypes.bfloat16

EXPB = -2.0  # exp(S + EXPB): softmax-invariant shift
LOG2E = 1.4426950408889634
C_ADJ = 0.04305  # Schraudolph minimax centering (HW f32->i16 rounds)
A16 = (1 << 23) * LOG2E / 65536.0
B16 = (127.0 - C_ADJ) * 128.0 + A16 * EXPB

# column layout of wqkv (output dims of the projection):
# m0 q01 (q_ha|q_hb) 0:128 | m1 k01 128:256 | m2 [q_hc|q_hc] 256:384
# m3 k2 384:448 | m4 v01 448:576 | m5 v2 576:640
MOFF = [0, 128, 256, 384, 448, 576]
MW = [128, 128, 128, 64, 128, 64]

ADD = mybir.AluOpType.add
MULT = mybir.AluOpType.mult


def build_nc(seq=N, n_dve=3, n_pool=0):
    # n_pool must stay 0: GPSIMD cannot read PSUM (BIR verifier).
    NS = seq // 512   # 512-wide query chunks
    NB = seq // 128   # 128-wide key blocks
    TILES = []
    b = 0
    while b < NB:
        n = min(3, NB - b)
        if NB - b == 4:
            n = 2  # avoid a trailing 1-block tile
        TILES.append(list(range(b, b + n)))
        b += n
    NT = len(TILES)

    # per-stream exp engine assignment: spread DVE/Pool tiles over t
    n_off = min(n_dve + n_pool, NT)
    off_ts = [int((i + 0.5) * NT / n_off) for i in range(n_off)] if n_off else []
    dve_set = set(off_ts[:n_dve])
    pool_set = set(off_ts[n_dve:])

    nc = bacc.Bacc("TRN2", target_bir_lowering=False, debug=False, num_devices=8)
    xt = nc.dram_tensor("xt", [768, seq], BF16, kind="ExternalInput").ap()
    wqkv = nc.dram_tensor("wqkv", [768, 640], BF16, kind="ExternalInput").ap()
    wb = nc.dram_tensor("wb", [128, 6], F32, kind="ExternalInput").ap()
    pwt = nc.dram_tensor("pwt", [384, 768], BF16, kind="ExternalInput").ap()
    vbias = nc.dram_tensor("vbias", [128, 192], F32, kind="ExternalInput").ap()
    y = nc.dram_tensor("y", [seq, 768], F32, kind="ExternalOutput").ap()

    with tile.TileContext(nc) as tc, ExitStack() as ctx:
        const = ctx.enter_context(tc.tile_pool(name="const", bufs=1))
        big = ctx.enter_context(tc.tile_pool(name="big", bufs=1))
        pt_pool = ctx.enter_context(tc.tile_pool(name="ptp", bufs=3))
        osb_pool = ctx.enter_context(tc.tile_pool(name="osb", bufs=3))
        rr_pool = ctx.enter_context(tc.tile_pool(name="rrp", bufs=2))
        ysb_pool = ctx.enter_context(tc.tile_pool(name="ysb", bufs=2))
        stp = ctx.enter_context(tc.tile_pool(name="stp", bufs=2, space="PSUM"))
        oax = ctx.enter_context(tc.tile_pool(name="oax", bufs=2, space="PSUM"))

        # ---- weights / constants (issued on sync queue) ----
        w_sb = []
        for cch in range(6):
            t = const.tile([128, 640], BF16, tag=f"w{cch}", name=f"w{cch}")
            nc.sync.dma_start(t[:], wqkv[cch * 128:(cch + 1) * 128, :])
            w_sb.append(t)
        wb_sb = const.tile([128, 6], F32, tag="wb")
        nc.sync.dma_start(wb_sb[:], wb[:])
        vb_sb = const.tile([128, 192], F32, tag="vb")
        nc.sync.dma_start(vb_sb[:], vbias[:])

        # x^T resident: xst[cch][s] = [128, 512] bf16, s-major
        xst = [[None] * NS for _ in range(6)]
        for s in range(NS):
            for cch in range(6):
                t = const.tile([128, 512], BF16, tag=f"x{cch}_{s}", name="xs")
                nc.sync.dma_start(
                    t[:], xt[cch * 128:(cch + 1) * 128, s * 512:(s + 1) * 512])
                xst[cch][s] = t
            if s == 1:
                pw_sb = [const.tile([128, 768], BF16, tag=f"pw{h}", name=f"pwt{h}")
                         for h in range(3)]
                for h in range(3):
                    nc.sync.dma_start(pw_sb[h][:], pwt[h * 128:(h + 1) * 128, :])

        ones_sb = const.tile([128, 64], F32R, tag="ones")
        nc.vector.memset(ones_sb[:].bitcast(F32), 1.0)
        expb_sb = const.tile([128, 1], F32, tag="expb")
        nc.vector.memset(expb_sb[:], EXPB)

        # ---- persistent tensors ----
        # memsets run FIRST on the otherwise-idle Pool queue (they gate the
        # first S matmuls / vaug bias-add; keep them off DVE/SP).
        q01 = big.tile([128, seq], BF16, tag="q01")
        q2 = big.tile([128, seq], BF16, tag="q2")
        ka = big.tile([128, seq], BF16, tag="ka")
        kb = big.tile([128, seq], BF16, tag="kb")
        kc = big.tile([128, seq], BF16, tag="kc")
        # vaug[:, h, blk, 0:64] = V block; col 64 = 1.0 (denominator row);
        # cols 65:79 pad (never read) -- only col 64 needs the memset.
        vaug = big.tile([128, 3, NB, 80], BF16, tag="va", name="va")
        nc.gpsimd.memset(vaug[:, :, :, 64:65], 1.0)
        nc.gpsimd.memset(ka[64:128, :].bitcast(mybir.dt.uint16), 0)
        nc.gpsimd.memset(kb[0:64, :].bitcast(mybir.dt.uint16), 0)
        nc.gpsimd.memset(kc[64:128, :].bitcast(mybir.dt.uint16), 0)
        otp = [big.tile([128, 512], BF16, tag=f"otp{h}", name=f"otp{h}")
               for h in range(3)]
        for h in range(3):
            nc.gpsimd.memset(otp[h][:].bitcast(mybir.dt.uint16), 0)

        heads = [(ka, q01), (kb, q01), (kc, q2)]
        streams = [(Q, h) for Q in range(NS) for h in range(3)]
        NSTR = len(streams)

        def copyback(dst, srcap, bias_ap):
            nc.vector.tensor_scalar_add(dst, srcap, bias_ap)

        # ---------- emission helpers ----------

        def emit_qproj_slice(Qn, mi):
            """mi 0 -> q01, 1 -> q2 (m-group 0 / 2)."""
            m = (0, 2)[mi]
            ps = stp.tile([128, 512], F32, tag="stp", name="psq")
            for cch in range(6):
                nc.tensor.matmul(
                    ps[:],
                    lhsT=w_sb[cch][:, MOFF[m]:MOFF[m] + 128],
                    rhs=xst[cch][Qn][:],
                    start=(cch == 0),
                    stop=(cch == 5),
                )
            qs = slice(Qn * 512, (Qn + 1) * 512)
            dst = (q01, q2)[mi]
            copyback(dst[:, qs], ps[:], wb_sb[:, (0, 2)[mi]:(0, 2)[mi] + 1])

        def emit_k01(s):
            ss = slice(s * 512, (s + 1) * 512)
            ps = stp.tile([128, 512], F32, tag="stp", name="psk")
            for cch in range(6):
                nc.tensor.matmul(
                    ps[:],
                    lhsT=w_sb[cch][:, MOFF[1]:MOFF[1] + 128],
                    rhs=xst[cch][s][:],
                    start=(cch == 0),
                    stop=(cch == 5),
                )
            copyback(ka[0:64, ss], ps[0:64, :], wb_sb[0:64, 1:2])
            copyback(kb[64:128, ss], ps[64:128, :], wb_sb[64:128, 1:2])

        def emit_k2(s):
            ss = slice(s * 512, (s + 1) * 512)
            ps = stp.tile([128, 512], F32, tag="stp", name="psk2")
            for cch in range(6):
                nc.tensor.matmul(
                    ps[0:64, :],
                    lhsT=w_sb[cch][:, MOFF[3]:MOFF[3] + 64],
                    rhs=xst[cch][s][:],
                    start=(cch == 0),
                    stop=(cch == 5),
                )
            copyback(kc[0:64, ss], ps[0:64, :], wb_sb[0:64, 3:4])

        def emit_vdir(s, j):
            """V projected directly as [128 keys, 192 dims(3 heads)]."""
            ps = stp.tile([128, 192], F32, tag="stp", name="psv")
            for cch in range(6):
                nc.tensor.matmul(
                    ps[:],
                    lhsT=xst[cch][s][:, j * 128:(j + 1) * 128],
                    rhs=w_sb[cch][:, 448:640],
                    start=(cch == 0),
                    stop=(cch == 5),
                )
            blk = 4 * s + j
            # free orders line up: ps/vb cols = [va|vb|vc], out = h-major
            nc.vector.tensor_tensor(
                vaug[:, :, blk:blk + 1, 0:64], ps[:], vb_sb[:], ADD)

        def emit_exp(ps, pt, nl, eng):
            if eng == "act":
                nc.scalar.activation(
                    pt[:, 0:nl, :], ps[:, 0:nl, :],
                    mybir.ActivationFunctionType.Exp, bias=expb_sb[:],
                )
            else:
                e = nc.vector if eng == "dve" else nc.gpsimd
                e.tensor_scalar(
                    pt[:, 0:nl, :].bitcast(I16), ps[:, 0:nl, :],
                    float(A16), float(B16), MULT, ADD,
                )

        def emit_av(oacc, h, pt, blocks, start, stop):
            n = len(blocks)
            for j, blk in enumerate(blocks):
                nc.tensor.matmul(
                    oacc[:],
                    lhsT=vaug[:, h:h + 1, blk:blk + 1, 0:65],
                    rhs=pt[:, j:j + 1, :],
                    start=(start and j == 0),
                    stop=(stop and j == n - 1),
                )

        def emit_yproj(Q, nt):
            ps = stp.tile([128, 2, 512], F32, tag="stp", name="psy")
            for sl, co, cw in ((0, 0, 512), (1, 512, 256)):
                for h in range(3):
                    nc.tensor.matmul(
                        ps[:, sl:sl + 1, 0:cw],
                        lhsT=otp[h][:, nt * 128:(nt + 1) * 128],
                        rhs=pw_sb[h][:, co:co + cw],
                        start=(h == 0),
                        stop=(h == 2),
                    )
            ysb = ysb_pool.tile([128, 768], F32, tag="ysb", name="ysb")
            nc.vector.tensor_copy(ysb[:, 0:512], ps[:, 0:1, :])
            nc.vector.tensor_copy(ysb[:, 512:768], ps[:, 1:2, 0:256])
            r0 = Q * 512 + nt * 128
            nc.sync.dma_start(y[r0:r0 + 128, :], ysb[:])

        def emit_norm(h, osb, rrr):
            psb = oax.tile([64, 512], F32, tag="o", name="psb")
            nc.tensor.matmul(
                psb[0:64, :],
                lhsT=ones_sb[64:65, 0:64],
                rhs=rrr[64:65, :],
                start=True,
                stop=True,
            )
            nc.vector.tensor_mul(otp[h][0:64, :], osb[0:64, :], psb[0:64, :])

        # ---------- flat pipeline driver ----------

        extras = deque()   # (kind, Q, closure)
        delayed = []       # (release_slot, kind, Q, closure)
        pend = deque()     # (si, t, pt)
        oaccs = {}
        gslot = [0]        # global tile-slot counter

        def release_delayed():
            for e in [e for e in delayed if e[0] <= gslot[0]]:
                delayed.remove(e)
                extras.append(e[1:])

        def pop_extra():
            release_delayed()
            if extras:
                extras.popleft()[2]()

        def run_qproj_now(Q):
            """Force any still-queued qproj(Q) extras before stream (Q, 0)."""
            keep = deque()
            while extras:
                e = extras.popleft()
                if e[0] == "qproj" and e[1] == Q:
                    e[2]()
                else:
                    keep.append(e)
            extras.extend(keep)

        def emit_stream_tile(si, t):
            Q, h = streams[si]
            kt, qt = heads[h]
            blocks = TILES[t]
            nl = len(blocks)
            qs = slice(Q * 512, (Q + 1) * 512)
            ps = stp.tile([128, nl, 512], F32, tag="stp", name="ps")
            for j, blk in enumerate(blocks):
                nc.tensor.matmul(
                    ps[:, j:j + 1, :],
                    lhsT=kt[:, blk * 128:(blk + 1) * 128],
                    rhs=qt[:, qs],
                    start=True,
                    stop=True,
                )
            pt = pt_pool.tile([128, 3, 512], BF16, tag="pt", name="pt")
            if si == 0:
                eng = "act"  # phase-1: DVE/Pool busy with copybacks
            else:
                eng = "dve" if t in dve_set else ("pool" if t in pool_set else "act")
            emit_exp(ps, pt, nl, eng)
            pend.append((si, t, pt))

        def retire_av():
            si, t, pt = pend.popleft()
            Q, h = streams[si]
            if t == 0:
                oaccs[si] = oax.tile([65, 512], F32, tag="o", name="oacc")
            emit_av(oaccs[si], h, pt, TILES[t], start=(t == 0), stop=(t == NT - 1))
            if t == NT - 1:
                end_stream(si)

        def end_stream(si):
            Q, h = streams[si]
            oacc = oaccs.pop(si)
            osb = osb_pool.tile([65, 512], F32, tag="osb", name="osb")
            # drain on ACT: frees the oacc PSUM slot without queueing behind
            # DVE's exp tiles
            nc.scalar.copy(osb[:], oacc[:])
            rr = rr_pool.tile([65, 512], F32, tag="rr", name="rr")
            nc.vector.reciprocal(rr[64:65, :], osb[64:65, :])
            rrr = rr_pool.tile([65, 512], F32R, tag="rrr", name="rrr")
            nc.vector.tensor_copy(rrr[64:65, :], rr[64:65, :])
            delayed.append((gslot[0] + 4, "norm", Q,
                            lambda h=h, o=osb, r=rrr: emit_norm(h, o, r)))
            if h == 2:
                # same release slot as norm(Q,2): FIFO keeps norm before
                # yproj (PE-queue order must match the DVE dep direction)
                for nt in range(4):
                    delayed.append(
                        (gslot[0] + 4, "yproj", Q,
                         lambda Q=Q, nt=nt: emit_yproj(Q, nt)))
            if h == 1 and Q + 1 < NS:
                for mi in range(2):
                    extras.append(
                        ("qproj", Q + 1,
                         lambda Q1=Q + 1, mi=mi: emit_qproj_slice(Q1, mi)))

        # phase 0: q projection for chunk 0
        emit_qproj_slice(0, 0)
        emit_qproj_slice(0, 1)

        # phase 1: k/v projection for all chunks, interleaved with stream 0
        t_next = 0

        def pull_stream0(s_done):
            nonlocal t_next
            while t_next < NT and TILES[t_next][-1] < 4 * s_done:
                if len(pend) >= 2:
                    retire_av()
                emit_stream_tile(0, t_next)
                t_next += 1
                gslot[0] += 1

        for s in range(NS):
            emit_k01(s)
            pull_stream0(s)      # keys of chunk s-1 usable
            emit_k2(s)
            for j in range(4):
                emit_vdir(s, j)
                pull_stream0(s)
        pull_stream0(NS)         # all keys ready: rest of stream 0

        # steady state: remaining streams
        for si in range(1, NSTR):
            Q, h = streams[si]
            if h == 0:
                run_qproj_now(Q)
            for t in range(NT):
                if len(pend) >= 2:
                    retire_av()   # AV(g-2) before S(g): pt is ready, S waits exp
                emit_stream_tile(si, t)
                gslot[0] += 1
                if t >= 1:
                    pop_extra()

        # tail: drain pipeline and remaining extras
        while pend:
            retire_av()
        gslot[0] += 8
        while extras or delayed:
            gslot[0] += 1
            pop_extra()

    nc.compile()
    return nc


def host_prep(x, qkv_w, qkv_b, proj_w, seq=N):
    """Build the 8 per-core input maps."""
    f = np.float32
    x = np.asarray(x, f)
    qkv_w = np.asarray(qkv_w, f)
    qkv_b = np.asarray(qkv_b, f)
    proj_w = np.asarray(proj_w, f)

    xts = [np.ascontiguousarray(x[b].T).astype(BF16NP) for b in range(B)]

    in_maps = []
    for core in range(8):
        b, g = core // 4, core % 4
        ha, hb_, hc = 3 * g, 3 * g + 1, 3 * g + 2

        def Wrow(base, h):
            return qkv_w[base + h * 64: base + (h + 1) * 64, :]  # [64, 768]

        def brow(base, h):
            return qkv_b[base + h * 64: base + (h + 1) * 64]

        cols = np.concatenate(
            [
                Wrow(0, ha).T * SCALE, Wrow(0, hb_).T * SCALE,   # q01
                Wrow(C, ha).T, Wrow(C, hb_).T,                   # k01 -> ka/kb
                Wrow(0, hc).T * SCALE, Wrow(0, hc).T * SCALE,    # q2 duplicated
                Wrow(C, hc).T,                                   # k2
                Wrow(2 * C, ha).T, Wrow(2 * C, hb_).T,           # v01
                Wrow(2 * C, hc).T,                               # v2
            ],
            axis=1,
        )  # [768, 640]
        bias = np.concatenate(
            [
                brow(0, ha) * SCALE, brow(0, hb_) * SCALE,
                brow(C, ha), brow(C, hb_),
                brow(0, hc) * SCALE, brow(0, hc) * SCALE,
                brow(C, hc),
                brow(2 * C, ha), brow(2 * C, hb_), brow(2 * C, hc),
            ]
        )  # [640]
        wbm = np.zeros((128, 6), f)
        for m in range(6):
            wbm[0:MW[m], m] = bias[MOFF[m]:MOFF[m] + MW[m]]
        vbm = np.zeros((128, 192), f)
        vbm[:, :] = bias[448:640][None, :]
        pwt = np.zeros((384, 768), f)
        for i, h in enumerate((ha, hb_, hc)):
            pwt[i * 128:i * 128 + 64, :] = proj_w.T[h * 64:(h + 1) * 64, :]

        in_maps.append(
            {
                "xt": xts[b][:, :seq],
                "wqkv": np.ascontiguousarray(cols).astype(BF16NP),
                "wb": wbm,
                "vbias": vbm,
                "pwt": pwt.astype(BF16NP),
            }
        )
    return in_maps


_nc_cache = {}


def _get_nc(seq=N):
    key = (seq,)
    if key not in _nc_cache:
        _nc_cache[key] = build_nc(seq)
    return _nc_cache[key]


def kernel(x, qkv_w, qkv_b, proj_w, proj_b, _trace=False):
    from concourse.bass_utils import run_bass_kernel_spmd

    nc = _get_nc()
    in_maps = host_prep(x, qkv_w, qkv_b, proj_w)
    res = run_bass_kernel_spmd(nc, in_maps, list(range(8)), trace=_trace)
    proj_b = np.asarray(proj_b, np.float32)
    out = np.zeros((B, N, C), np.float32)
    for b in range(B):
        acc = np.zeros((N, C), np.float32)
        for g in range(4):
            acc += res.results[b * 4 + g]["y"]
        out[b] = acc + proj_b[None, :]
    if _trace:
        return out, res
    return out


# revision 23
# speedup vs baseline: 1.1557x; 1.0892x over previous
"""Fused multi-head attention block (B=2, N=4096, C=768, H=12, D=64) for 8
Trainium2 NeuronCores.

Sharding: core c -> (batch b = c // 4, head-group g = c % 4, heads
[3g, 3g+1, 3g+2]).  Megatron-style: qkv weights column-split per head
group, proj weights row-split; each core emits a partial [N, C] output
and the host sums the 4 partials per batch and adds proj_b.

Per-core kernel v3 (flat-pipeline design):
  - x^T and all weights SBUF-resident in bf16.
  - One global software pipeline over all (chunk, head) attention
    streams: per 512-query x 384-key tile, PE emits S(g) then AV(g-2),
    with the AV drain of stream k interleaved with the first S tiles of
    stream k+1 (oacc ring of 2 PSUM banks) -- no PE bubble at head
    boundaries.
  - exp(S) split across THREE engines: ACT (exact exp instruction) for
    most tiles, DVE + GPSIMD via one-instruction Schraudolph
    (int16 <- A*S + B; the int16 IS the bf16 bit pattern of ~exp(S),
    max rel err ~3% sawtooth which largely cancels in the softmax
    ratio; end-to-end rel err ~1.25e-2, gate 2e-2).  ACT alone was the
    456us critical path in v2.
  - V^T never materialized: V is projected directly in [keys, dims]
    layout (lhsT = x^T block, rhs = W_v columns) into vaug, killing the
    PE transpose passes and their DVE copybacks.
  - Softmax denominators: ones-column 64 of vaug -> DVE reciprocal ->
    PE broadcast (f32r outer product) -> DVE multiply; the broadcast
    PSUM tile shares the oacc ring.
  - qproj / yproj / normalize / phase-1 slices are woven one-per-tile
    into the pipeline as 'extras' to fill the exp-latency gap on PE.
"""

import sys

sys.path.insert(0, "/opt/trn_rl_repo")

from collections import deque
from contextlib import ExitStack

import numpy as np
import ml_dtypes

import concourse.bacc as bacc
import concourse.bass as bass
import concourse.mybir as mybir
import concourse.tile as tile

B, N, C, H, D = 2, 4096, 768, 12, 64
SCALE = D ** -0.5
F32 = mybir.dt.float32
F32R = mybir.dt.float32r
BF16 = mybir.dt.bfloat16
I16 = mybir.dt.int16
BF16NP = ml_dtypes.bfloat16

EXPB = -2.0  # exp(S + EXPB): softmax-invariant shift
LOG2E = 1.4426950408889634
C_ADJ = 0.04305  # Schraudolph minimax centering (HW f32->i16 rounds)
A16 = (1 << 23) * LOG2E / 65536.0
B16 = (127.0 - C_ADJ) * 128.0 + A16 * EXPB

# column layout of wqkv (output dims of the projection):
# m0 q01 (q_ha|q_hb) 0:128 | m1 k01 128:256 | m2 [q_hc|q_hc] 256:384
# m3 k2 384:448 | m4 v01 448:576 | m5 v2 576:640
MOFF = [0, 128, 256, 384, 448, 576]
MW = [128, 128, 128, 64, 128, 64]

ADD = mybir.AluOpType.add
MULT = mybir.AluOpType.mult


def build_nc(seq=N, n_dve=3, n_pool=0):
    # n_pool must stay 0: GPSIMD cannot read PSUM (BIR verifier).
    NS = seq // 512   # 512-wide query chunks
    NB = seq // 128   # 128-wide key blocks
    TILES = []
    b = 0
    while b < NB:
        n = min(3, NB - b)
        if NB - b == 4:
            n = 2  # avoid a trailing 1-block tile
        TILES.append(list(range(b, b + n)))
        b += n
    NT = len(TILES)

    # per-stream exp engine assignment: spread DVE/Pool tiles over t
    n_off = min(n_dve + n_pool, NT)
    off_ts = [int((i + 0.5) * NT / n_off) for i in range(n_off)] if n_off else []
    dve_set = set(off_ts[:n_dve])
    pool_set = set(off_ts[n_dve:])

    nc = bacc.Bacc("TRN2", target_bir_lowering=False, debug=False, num_devices=8)
    xt = nc.dram_tensor("xt", [768, seq], BF16, kind="ExternalInput").ap()
    wqkv = nc.dram_tensor("wqkv", [768, 640], BF16, kind="ExternalInput").ap()
    wb = nc.dram_tensor("wb", [128, 12], F32, kind="ExternalInput").ap()
    pwt = nc.dram_tensor("pwt", [384, 768], BF16, kind="ExternalInput").ap()
    vbias = nc.dram_tensor("vbias", [128, 192], F32, kind="ExternalInput").ap()
    y = nc.dram_tensor("y", [seq, 768], F32, kind="ExternalOutput").ap()

    with tile.TileContext(nc) as tc, ExitStack() as ctx:
        const = ctx.enter_context(tc.tile_pool(name="const", bufs=1))
        big = ctx.enter_context(tc.tile_pool(name="big", bufs=1))
        pt_pool = ctx.enter_context(tc.tile_pool(name="ptp", bufs=3))
        osb_pool = ctx.enter_context(tc.tile_pool(name="osb", bufs=3))
        rr_pool = ctx.enter_context(tc.tile_pool(name="rrp", bufs=2))
        ysb_pool = ctx.enter_context(tc.tile_pool(name="ysb", bufs=2))
        # PSUM: stp 2x3 banks (S tiles + phase-1 slices), oacc 1 bank,
        # tx 1 bank (psb / qproj / yproj halves) = 8 banks total
        stp = ctx.enter_context(tc.tile_pool(name="stp", bufs=2, space="PSUM"))
        oax = ctx.enter_context(tc.tile_pool(name="oax", bufs=1, space="PSUM"))
        txp = ctx.enter_context(tc.tile_pool(name="txp", bufs=1, space="PSUM"))

        # ---- weights / constants (issued on sync queue) ----
        w_sb = []
        for cch in range(6):
            t = const.tile([128, 640], BF16, tag=f"w{cch}", name=f"w{cch}")
            nc.sync.dma_start(t[:], wqkv[cch * 128:(cch + 1) * 128, :])
            w_sb.append(t)
        wb_sb = const.tile([128, 12], F32, tag="wb")
        nc.sync.dma_start(wb_sb[:], wb[:])
        vb_sb = const.tile([128, 192], F32, tag="vb")
        nc.sync.dma_start(vb_sb[:], vbias[:])

        # ---- persistent tensors; memsets lead the gpsimd queue ----
        q01 = big.tile([128, seq], BF16, tag="q01")
        q2 = big.tile([128, seq], BF16, tag="q2")
        ka = big.tile([128, seq], BF16, tag="ka")
        kb = big.tile([128, seq], BF16, tag="kb")
        kc = big.tile([128, seq], BF16, tag="kc")
        # ka/kb dead halves are zeroed by the masked copyback; kc needs a
        # one-time zero of its dead half.
        nc.gpsimd.memset(kc[64:128, :].bitcast(mybir.dt.uint16), 0)
        # vaug[:, h, blk, 0:64] = V block; col 64 = 1.0 (denominator row);
        # cols 65:79 pad (never read) -- only col 64 needs the memset.
        vaug = big.tile([128, 3, NB, 80], BF16, tag="va", name="va")
        nc.gpsimd.memset(vaug[:, :, :, 64:65], 1.0)
        otp = [big.tile([128, 512], BF16, tag=f"otp{h}", name=f"otp{h}")
               for h in range(3)]
        for h in range(3):
            nc.gpsimd.memset(otp[h][:].bitcast(mybir.dt.uint16), 0)

        # x^T resident: xst[cch][s] = [128, 512] bf16, s-major; issue split
        # across the sync and gpsimd queues (issue cost ~565ns each)
        xst = [[None] * NS for _ in range(6)]
        for s in range(NS):
            eng = nc.sync if s % 2 == 0 else nc.gpsimd
            for cch in range(6):
                t = const.tile([128, 512], BF16, tag=f"x{cch}_{s}", name="xs")
                eng.dma_start(
                    t[:], xt[cch * 128:(cch + 1) * 128, s * 512:(s + 1) * 512])
                xst[cch][s] = t
            if s == 1:
                pw_sb = [const.tile([128, 768], BF16, tag=f"pw{h}", name=f"pwt{h}")
                         for h in range(3)]
                for h in range(3):
                    nc.sync.dma_start(pw_sb[h][:], pwt[h * 128:(h + 1) * 128, :])

        ones_sb = const.tile([128, 64], F32R, tag="ones")
        nc.vector.memset(ones_sb[:].bitcast(F32), 1.0)
        expb_sb = const.tile([128, 1], F32, tag="expb")
        nc.vector.memset(expb_sb[:], EXPB)

        heads = [(ka, q01), (kb, q01), (kc, q2)]
        streams = [(Q, h) for Q in range(NS) for h in range(3)]
        NSTR = len(streams)

        def copyback(dst, srcap, bias_ap):
            nc.vector.tensor_scalar_add(dst, srcap, bias_ap)

        # ---------- emission helpers ----------

        def emit_qproj_slice(Qn, mi):
            """mi 0 -> q01, 1 -> q2 (m-group 0 / 2)."""
            m = (0, 2)[mi]
            ps = txp.tile([128, 512], F32, tag="tx", name="psq")
            for cch in range(6):
                nc.tensor.matmul(
                    ps[:],
                    lhsT=w_sb[cch][:, MOFF[m]:MOFF[m] + 128],
                    rhs=xst[cch][Qn][:],
                    start=(cch == 0),
                    stop=(cch == 5),
                )
            qs = slice(Qn * 512, (Qn + 1) * 512)
            dst = (q01, q2)[mi]
            copyback(dst[:, qs], ps[:], wb_sb[:, (0, 2)[mi]:(0, 2)[mi] + 1])

        def emit_k01(s):
            ss = slice(s * 512, (s + 1) * 512)
            ps = stp.tile([128, 512], F32, tag="stp", name="psk")
            for cch in range(6):
                nc.tensor.matmul(
                    ps[:],
                    lhsT=w_sb[cch][:, MOFF[1]:MOFF[1] + 128],
                    rhs=xst[cch][s][:],
                    start=(cch == 0),
                    stop=(cch == 5),
                )
            # masked copybacks write the full 128 partitions (dead half
            # times 0 plus 0 bias) -- avoids the big one-time zero memsets
            nc.vector.tensor_scalar(
                ka[:, ss], ps[:], wb_sb[:, 6:7], wb_sb[:, 8:9], MULT, ADD)
            nc.vector.tensor_scalar(
                kb[:, ss], ps[:], wb_sb[:, 7:8], wb_sb[:, 9:10], MULT, ADD)

        def emit_k2(s):
            ss = slice(s * 512, (s + 1) * 512)
            ps = stp.tile([128, 512], F32, tag="stp", name="psk2")
            for cch in range(6):
                nc.tensor.matmul(
                    ps[0:64, :],
                    lhsT=w_sb[cch][:, MOFF[3]:MOFF[3] + 64],
                    rhs=xst[cch][s][:],
                    start=(cch == 0),
                    stop=(cch == 5),
                )
            copyback(kc[0:64, ss], ps[0:64, :], wb_sb[0:64, 3:4])

        def emit_vdir(s, j):
            """V projected directly as [128 keys, 192 dims(3 heads)]."""
            ps = stp.tile([128, 192], F32, tag="stp", name="psv")
            for cch in range(6):
                nc.tensor.matmul(
                    ps[:],
                    lhsT=xst[cch][s][:, j * 128:(j + 1) * 128],
                    rhs=w_sb[cch][:, 448:640],
                    start=(cch == 0),
                    stop=(cch == 5),
                )
            blk = 4 * s + j
            # free orders line up: ps/vb cols = [va|vb|vc], out = h-major
            nc.vector.tensor_tensor(
                vaug[:, :, blk:blk + 1, 0:64], ps[:], vb_sb[:], ADD)

        def emit_exp(ps, pt, nl, eng):
            if eng == "act":
                nc.scalar.activation(
                    pt[:, 0:nl, :], ps[:, 0:nl, :],
                    mybir.ActivationFunctionType.Exp, bias=expb_sb[:],
                )
            else:
                e = nc.vector if eng == "dve" else nc.gpsimd
                e.tensor_scalar(
                    pt[:, 0:nl, :].bitcast(I16), ps[:, 0:nl, :],
                    float(A16), float(B16), MULT, ADD,
                )

        def emit_av(oacc, h, pt, blocks, start, stop):
            n = len(blocks)
            for j, blk in enumerate(blocks):
                nc.tensor.matmul(
                    oacc[:],
                    lhsT=vaug[:, h:h + 1, blk:blk + 1, 0:65],
                    rhs=pt[:, j:j + 1, :],
                    start=(start and j == 0),
                    stop=(stop and j == n - 1),
                )

        ysb_live = {}

        def emit_yproj_half(Q, nt, half):
            """half 0: cols 0:512, half 1: cols 512:768 (+ y DMA)."""
            co, cw = (0, 512) if half == 0 else (512, 256)
            ps = txp.tile([128, cw], F32, tag="tx", name="psy")
            for h in range(3):
                nc.tensor.matmul(
                    ps[:],
                    lhsT=otp[h][:, nt * 128:(nt + 1) * 128],
                    rhs=pw_sb[h][:, co:co + cw],
                    start=(h == 0),
                    stop=(h == 2),
                )
            if half == 0:
                ysb = ysb_pool.tile([128, 768], F32, tag="ysb", name="ysb")
                ysb_live[(Q, nt)] = ysb
            else:
                ysb = ysb_live.pop((Q, nt))
            nc.vector.tensor_copy(ysb[:, co:co + cw], ps[:])
            if half == 1:
                r0 = Q * 512 + nt * 128
                nc.sync.dma_start(y[r0:r0 + 128, :], ysb[:])

        def emit_norm(h, osb, rrr):
            psb = txp.tile([64, 512], F32, tag="tx", name="psb")
            nc.tensor.matmul(
                psb[0:64, :],
                lhsT=ones_sb[64:65, 0:64],
                rhs=rrr[64:65, :],
                start=True,
                stop=True,
            )
            nc.vector.tensor_mul(otp[h][0:64, :], osb[0:64, :], psb[0:64, :])

        # ---------- flat pipeline driver ----------

        extras = deque()   # (kind, Q, closure)
        delayed = []       # (release_slot, kind, Q, closure)
        pend = deque()     # (si, t, pt)
        oaccs = {}
        gslot = [0]        # global tile-slot counter

        def release_delayed():
            for e in [e for e in delayed if e[0] <= gslot[0]]:
                delayed.remove(e)
                extras.append(e[1:])

        def pop_extra():
            release_delayed()
            if extras:
                extras.popleft()[2]()

        def run_qproj_now(Q):
            """Force any still-queued qproj(Q) extras before stream (Q, 0)."""
            keep = deque()
            while extras:
                e = extras.popleft()
                if e[0] == "qproj" and e[1] == Q:
                    e[2]()
                else:
                    keep.append(e)
            extras.extend(keep)

        def emit_stream_tile(si, t):
            Q, h = streams[si]
            kt, qt = heads[h]
            blocks = TILES[t]
            nl = len(blocks)
            qs = slice(Q * 512, (Q + 1) * 512)
            ps = stp.tile([128, nl, 512], F32, tag="stp", name="ps")
            for j, blk in enumerate(blocks):
                nc.tensor.matmul(
                    ps[:, j:j + 1, :],
                    lhsT=kt[:, blk * 128:(blk + 1) * 128],
                    rhs=qt[:, qs],
                    start=True,
                    stop=True,
                )
            pt = pt_pool.tile([128, 3, 512], BF16, tag="pt", name="pt")
            if si == 0:
                eng = "act"  # phase-1: DVE/Pool busy with copybacks
            else:
                eng = "dve" if t in dve_set else ("pool" if t in pool_set else "act")
            emit_exp(ps, pt, nl, eng)
            pend.append((si, t, pt))

        def retire_av():
            si, t, pt = pend.popleft()
            Q, h = streams[si]
            if t == 0:
                oaccs[si] = oax.tile([65, 512], F32, tag="o", name="oacc")
            emit_av(oaccs[si], h, pt, TILES[t], start=(t == 0), stop=(t == NT - 1))
            if t == NT - 1:
                end_stream(si)

        def end_stream(si):
            Q, h = streams[si]
            oacc = oaccs.pop(si)
            osb = osb_pool.tile([65, 512], F32, tag="osb", name="osb")
            # drain on ACT: frees the oacc PSUM slot without queueing behind
            # DVE's exp tiles
            nc.scalar.copy(osb[:], oacc[:])
            rr = rr_pool.tile([65, 512], F32, tag="rr", name="rr")
            nc.vector.reciprocal(rr[64:65, :], osb[64:65, :])
            rrr = rr_pool.tile([65, 512], F32R, tag="rrr", name="rrr")
            nc.vector.tensor_copy(rrr[64:65, :], rr[64:65, :])
            delayed.append((gslot[0] + 6, "norm", Q,
                            lambda h=h, o=osb, r=rrr: emit_norm(h, o, r)))
            if h == 2:
                # same release slot as norm(Q,2): FIFO keeps norm before
                # yproj (PE-queue order must match the DVE dep direction)
                for nt in range(4):
                    for half in range(2):
                        delayed.append(
                            (gslot[0] + 6, "yproj", Q,
                             lambda Q=Q, nt=nt, hf=half:
                             emit_yproj_half(Q, nt, hf)))
            if h == 1 and Q + 1 < NS:
                for mi in range(2):
                    extras.append(
                        ("qproj", Q + 1,
                         lambda Q1=Q + 1, mi=mi: emit_qproj_slice(Q1, mi)))

        # phase 0: q projection for chunk 0
        emit_qproj_slice(0, 0)
        emit_qproj_slice(0, 1)

        # phase 1: k/v projection for all chunks, interleaved with stream 0
        t_next = 0

        def pull_stream0(s_done):
            nonlocal t_next
            while t_next < NT and TILES[t_next][-1] < 4 * s_done:
                if len(pend) >= 2:
                    retire_av()
                emit_stream_tile(0, t_next)
                t_next += 1
                gslot[0] += 1

        for s in range(NS):
            emit_k01(s)
            pull_stream0(s)      # keys of chunk s-1 usable
            emit_k2(s)
            for j in range(4):
                emit_vdir(s, j)
                pull_stream0(s)
        pull_stream0(NS)         # all keys ready: rest of stream 0

        # steady state: remaining streams
        for si in range(1, NSTR):
            Q, h = streams[si]
            if h == 0:
                run_qproj_now(Q)
            for t in range(NT):
                if len(pend) >= 2:
                    retire_av()   # AV(g-2) before S(g): pt is ready, S waits exp
                emit_stream_tile(si, t)
                gslot[0] += 1
                if t >= 1:
                    pop_extra()

        # tail: drain pipeline and remaining extras
        while pend:
            retire_av()
        gslot[0] += 8
        while extras or delayed:
            gslot[0] += 1
            pop_extra()

    nc.compile()
    return nc


def host_prep(x, qkv_w, qkv_b, proj_w, seq=N):
    """Build the 8 per-core input maps."""
    f = np.float32
    x = np.asarray(x, f)
    qkv_w = np.asarray(qkv_w, f)
    qkv_b = np.asarray(qkv_b, f)
    proj_w = np.asarray(proj_w, f)

    xts = [np.ascontiguousarray(x[b].T).astype(BF16NP) for b in range(B)]

    in_maps = []
    for core in range(8):
        b, g = core // 4, core % 4
        ha, hb_, hc = 3 * g, 3 * g + 1, 3 * g + 2

        def Wrow(base, h):
            return qkv_w[base + h * 64: base + (h + 1) * 64, :]  # [64, 768]

        def brow(base, h):
            return qkv_b[base + h * 64: base + (h + 1) * 64]

        cols = np.concatenate(
            [
                Wrow(0, ha).T * SCALE, Wrow(0, hb_).T * SCALE,   # q01
                Wrow(C, ha).T, Wrow(C, hb_).T,                   # k01 -> ka/kb
                Wrow(0, hc).T * SCALE, Wrow(0, hc).T * SCALE,    # q2 duplicated
                Wrow(C, hc).T,                                   # k2
                Wrow(2 * C, ha).T, Wrow(2 * C, hb_).T,           # v01
                Wrow(2 * C, hc).T,                               # v2
            ],
            axis=1,
        )  # [768, 640]
        bias = np.concatenate(
            [
                brow(0, ha) * SCALE, brow(0, hb_) * SCALE,
                brow(C, ha), brow(C, hb_),
                brow(0, hc) * SCALE, brow(0, hc) * SCALE,
                brow(C, hc),
                brow(2 * C, ha), brow(2 * C, hb_), brow(2 * C, hc),
            ]
        )  # [640]
        wbm = np.zeros((128, 12), f)
        for m in range(6):
            wbm[0:MW[m], m] = bias[MOFF[m]:MOFF[m] + MW[m]]
        wbm[0:64, 6] = 1.0    # maskLo (ka copyback)
        wbm[64:128, 7] = 1.0  # maskHi (kb copyback)
        wbm[0:64, 8] = bias[128:192]    # b_k(ha), dead half zero
        wbm[64:128, 9] = bias[192:256]  # b_k(hb), dead half zero
        vbm = np.zeros((128, 192), f)
        vbm[:, :] = bias[448:640][None, :]
        pwt = np.zeros((384, 768), f)
        for i, h in enumerate((ha, hb_, hc)):
            pwt[i * 128:i * 128 + 64, :] = proj_w.T[h * 64:(h + 1) * 64, :]

        in_maps.append(
            {
                "xt": xts[b][:, :seq],
                "wqkv": np.ascontiguousarray(cols).astype(BF16NP),
                "wb": wbm,
                "vbias": vbm,
                "pwt": pwt.astype(BF16NP),
            }
        )
    return in_maps


_nc_cache = {}


def _get_nc(seq=N):
    key = (seq,)
    if key not in _nc_cache:
        _nc_cache[key] = build_nc(seq)
    return _nc_cache[key]


def kernel(x, qkv_w, qkv_b, proj_w, proj_b, _trace=False):
    from concourse.bass_utils import run_bass_kernel_spmd

    nc = _get_nc()
    in_maps = host_prep(x, qkv_w, qkv_b, proj_w)
    res = run_bass_kernel_spmd(nc, in_maps, list(range(8)), trace=_trace)
    proj_b = np.asarray(proj_b, np.float32)
    out = np.zeros((B, N, C), np.float32)
    for b in range(B):
        acc = np.zeros((N, C), np.float32)
        for g in range(4):
            acc += res.results[b * 4 + g]["y"]
        out[b] = acc + proj_b[None, :]
    if _trace:
        return out, res
    return out
